# revision 22
# baseline (speedup 1.0000x reference)
# MoE-routing kernel for Trainium2: out[b] = x[b] @ weight[y[b]] + bias[y[b]]
# x: [1024, 64, 1152] f32, y: [1024] int64, weight: [1000, 1152, 128] f32,
# bias: [1000, 128] f32 -> out: [1024, 64, 128] f32.
#
# The kernel is HBM-bound (all 8 cores together sit at the chip DMA
# roofline), so everything is organized around minimizing bytes moved:
#  - Data-parallel over batch (128 samples/core), with the routing gather
#    deduplicated: samples sharing a class form a "group" (size 1-4) whose
#    weight is loaded from HBM once. Group-size multisets are balanced so
#    all 8 cores run one SPMD program; per-y plans compile once and cache.
#  - Weights travel as bf16 except the first MF8=4 of 9 k-tiles, which go
#    as fp8e4m3 scaled by 1024 (the matching x k-tiles are pre-divided by
#    1024 in bf16, an exact exponent shift, so fp32 PSUM accumulation needs
#    no correction; mixed fp8xbf16 matmul operands are legal on trn2).
#    Measured output rel-err 1.79e-2 against the 2e-2 gate.
#  - Groups pack into bins (<=8 samples) with a small ramp at both ends;
#    bins pair into chunks and each chunk's x + bf16-w + fp8-w travel as
#    ONE contiguous DRAM param on a single queue (DMA issue costs ~0.5us
#    of queue time each; two busy queues contend and run slower than one).
#    fp8 bytes ride packed in bf16 columns and are bitcast back on device.
#  - Per group: 9 accumulating K=128 matmuls, weight k-tile stationary
#    [128,128], group's x moving [128, g*64], fp32 PSUM [128(out), g*64],
#    cast to bf16, stored o-major. Host adds bias and un-permutes.

import numpy as np
from collections import defaultdict

B, N, HIDDEN = 1024, 64, 1152
NUM_CLASSES = 1000
OUT_DIM = 128
KT = HIDDEN // 128  # 9 k-tiles
NCORES = 8
S = B // NCORES  # 128 samples per core
GMAX = 4         # max samples per class-group
BINMAX = 8       # max samples per DMA bin
MF8 = 4          # leading k-tiles whose weights go over HBM as fp8e4m3
WSC = 1024.0     # fp8 weight scale (x k-tiles pre-divided by WSC in bf16)

_cache = {}


def _make_template_and_groups(y):
    """Plan the computation. Returns (template, core_bins):
    template: tuple of bins; each bin is a tuple of group sizes (identical
      structure on every core -> one SPMD program).
    core_bins: [core][bin][group] -> (cls, [sample indices]) matching template.
    """
    by_class = defaultdict(list)
    for i, c in enumerate(np.asarray(y).astype(np.int64).tolist()):
        by_class[c].append(i)
    groups = []  # (cls, samples) with len(samples) <= GMAX
    for c in sorted(by_class):
        idxs = by_class[c]
        for j in range(0, len(idxs), GMAX):
            groups.append((c, idxs[j : j + GMAX]))

    def split_some(size, parts, want):
        # split `want` groups of `size` into `parts`; returns how many done
        done = 0
        i = 0
        while done < want and i < len(groups):
            c, s = groups[i]
            if len(s) == size:
                rep, o = [], 0
                for p in parts:
                    rep.append((c, s[o : o + p]))
                    o += p
                groups[i : i + 1] = rep
                done += 1
                i += len(rep)
            else:
                i += 1
        return done

    # make the count of each group size divisible by NCORES by splitting
    for size, parts in ((4, (2, 2)), (3, (2, 1)), (2, (1, 1))):
        n = sum(1 for _, s in groups if len(s) == size)
        r = n % NCORES
        if r:
            split_some(size, parts, r if n >= NCORES else n)
    cnt = [0] * (GMAX + 1)
    for _, s in groups:
        cnt[len(s)] += 1
    assert all(c % NCORES == 0 for c in cnt[1:]), cnt
    assert sum(k * c for k, c in enumerate(cnt)) == B

    # deal round-robin per size -> identical per-core multisets
    core_by_size = [defaultdict(list) for _ in range(NCORES)]
    dealt = defaultdict(int)
    for g in groups:
        k = len(g[1])
        core_by_size[dealt[k] % NCORES][k].append(g)
        dealt[k] += 1

    # build the shared bin template from the per-core size counts
    avail = {k: cnt[k] // NCORES for k in range(1, GMAX + 1)}

    def take_near(t):
        for k in range(min(t, GMAX), 0, -1):
            if avail.get(k, 0):
                avail[k] -= 1
                return k
        for k in range(t + 1, GMAX + 1):
            if avail.get(k, 0):
                avail[k] -= 1
                return k
        return None

    ramp = [take_near(t) for t in (1, 1, 2, 4)]
    ramp = [(k,) for k in ramp if k is not None]
    tail = [take_near(t) for t in (2, 1, 1)]
    tail = [(k,) for k in tail if k is not None]
    # middle: first-fit-decreasing into bins of <= BINMAX samples
    items = []
    for k in sorted(avail, reverse=True):
        items += [k] * avail[k]
    bins = []
    for it in items:
        for b in bins:
            if sum(b) + it <= BINMAX:
                b.append(it)
                break
        else:
            bins.append([it])
    # interleave PE-heavy (many-group) and PE-light bins to smooth the
    # compute/DMA demand mix through the pipeline
    bins.sort(key=len)
    lo, hi = 0, len(bins) - 1
    mid = []
    while lo <= hi:
        mid.append(bins[hi]); hi -= 1
        if lo <= hi:
            mid.append(bins[lo]); lo += 1
    template = tuple(tuple(b) for b in (list(ramp) + mid + list(tail)))

    # each core fills the template from its own per-size group lists
    core_bins = []
    for c in range(NCORES):
        filled = []
        for b in template:
            filled.append([core_by_size[c][k].pop() for k in b])
        core_bins.append(filled)
    return template, core_bins


def _build_nc(template):
    import concourse.bass as bass
    import concourse.mybir as mybir
    from concourse.tile import TileContext

    nc = bass.Bass()
    f32 = mybir.dt.float32
    bf16 = mybir.dt.bfloat16
    f8 = mybir.dt.float8e4
    KB = KT - MF8  # bf16 k-tiles per group

    # bins are paired into chunks; each chunk's x + bf16-w + fp8-w travel as
    # ONE contiguous bf16 DRAM param (fp8 bytes packed pairwise into bf16
    # columns and bitcast back on device): 11 input DMAs total
    chunks = [tuple(template[i : i + 2]) for i in range(0, len(template), 2)]

    def chunk_cols(ch):
        xc = sum(sum(b) for b in ch) * KT * N
        wc = sum(len(b) for b in ch) * KB * OUT_DIM
        vc = sum(len(b) for b in ch) * MF8 * OUT_DIM // 2
        return xc, wc, vc

    Cds, Ods = [], []
    for ci, ch in enumerate(chunks):
        xc, wc, vc = chunk_cols(ch)
        Cds.append(nc.declare_dram_parameter(f"c{ci}", [128, xc + wc + vc], bf16, isOutput=False))
        Ods.append(nc.declare_dram_parameter(f"o{ci}", [128, sum(sum(b) for b in ch) * N], bf16, isOutput=True))
    maxcols = max(sum(chunk_cols(ch)) for ch in chunks)
    maxout = max(sum(sum(b) for b in ch) for ch in chunks) * N

    with TileContext(nc) as tc:
        with (
            tc.tile_pool(name="cp", bufs=3) as cp,
            tc.tile_pool(name="op", bufs=3) as op,
            tc.tile_pool(name="pp", bufs=8, space="PSUM") as pp,
        ):
            for ci, ch in enumerate(chunks):
                xc, wc, vc = chunk_cols(ch)
                ct = cp.tile([128, maxcols], bf16, tag="ct")
                nc.sync.dma_start(out=ct[:, : xc + wc + vc], in_=Cds[ci][:, :])
                vap = ct[:, xc + wc : xc + wc + vc].bitcast(f8)
                ot = op.tile([128, maxout], bf16, tag="ot")
                xoff = 0      # x cols consumed within chunk
                goff = 0      # groups consumed within chunk
                ooff = 0      # out cols within chunk
                for b in ch:
                    bs = sum(b)
                    o = 0
                    for j, g in enumerate(b):
                        jj = goff + j
                        ps = pp.tile([128, GMAX * N], f32)
                        for k in range(KT):
                            if k < MF8:
                                lhsT = vap[:, (jj * MF8 + k) * OUT_DIM : (jj * MF8 + k + 1) * OUT_DIM]
                            else:
                                lhsT = ct[:, xc + (jj * KB + k - MF8) * OUT_DIM : xc + (jj * KB + k - MF8 + 1) * OUT_DIM]
                            nc.tensor.matmul(
                                ps[:, : g * N],
                                lhsT,
                                ct[:, xoff + (k * bs + o) * N : xoff + (k * bs + o + g) * N],
                                start=(k == 0),
                                stop=(k == KT - 1),
                            )
                        nc.vector.tensor_copy(ot[:, ooff + o * N : ooff + (o + g) * N], ps[:, : g * N])
                        o += g
                    xoff += bs * KT * N
                    goff += len(b)
                    ooff += bs * N
                nc.scalar.dma_start(out=Ods[ci][:, :], in_=ot[:, : ooff])

    _split_excess_waits(nc)
    nc.finalize()
    _split_excess_waits(nc)
    return nc


def _split_excess_waits(nc, max_waits=1):
    # walrus codegen rejects instructions with >max sync waits; Tile's tail
    # drain can carry several. Hoist the excess onto preceding no-ops.
    import concourse.mybir as mybir

    for f in nc.m.functions:
        for b in f.blocks:
            i = 0
            while i < len(b.instructions):
                inst = b.instructions[i]
                si = inst.sync_info
                if si is not None and len(si.on_wait) > max_waits:
                    excess = list(si.on_wait[:-max_waits])
                    si.on_wait = list(si.on_wait[-max_waits:])
                    for w in excess:
                        nop = mybir.InstNoOp(
                            name=nc.get_next_instruction_name(),
                            engine=inst.engine,
                            sync_info=mybir.SyncInfo(on_wait=[w], on_update=[]),
                            bass_nofuse=True,
                        )
                        nc.register_instruction(nop)
                        b.instructions.insert(i, nop)
                        i += 1
                i += 1


def kernel(x, y, weight, bias):
    import ml_dtypes
    from concourse.bass_utils import run_bass_kernel_spmd

    bf16 = ml_dtypes.bfloat16
    f8e4 = ml_dtypes.float8_e4m3
    x = np.ascontiguousarray(x, dtype=np.float32)
    weight = np.ascontiguousarray(weight, dtype=np.float32)
    yi = np.asarray(y).astype(np.int64)

    template, core_bins = _make_template_and_groups(yi)
    key = ("nc", template)
    if key not in _cache:
        _cache[key] = _build_nc(template)
    nc = _cache[key]
    NG = sum(len(b) for b in template)

    # x[s, n, k*128+p] -> Xt[s, p, k, n], bf16; fp8 k-tiles pre-divided by WSC
    # (exact exponent shift in bf16) to cancel the fp8 weight scale in PSUM
    x = x.copy()
    x[:, :, : MF8 * 128] *= np.float32(1.0 / WSC)
    Xt = np.ascontiguousarray(
        x.reshape(B, N, KT, 128).transpose(0, 3, 2, 1)
    ).astype(bf16)

    in_maps = []
    core_samples = []
    for c in range(NCORES):
        samples = [i for b in core_bins[c] for _, gss in b for i in gss]
        assert len(samples) == S
        core_samples.append(samples)
        m = {}
        cbins = core_bins[c]
        for ci in range(0, len(cbins), 2):
            ch = cbins[ci : ci + 2]
            xparts, wparts, vparts = [], [], []
            for b in ch:
                ss = [i for _, gss in b for i in gss]
                bs, nw = len(ss), len(b)
                xparts.append(
                    np.ascontiguousarray(Xt[ss].transpose(1, 2, 0, 3)).reshape(
                        128, bs * KT * N
                    )
                )
                wsel = weight[[cls for cls, _ in b]].reshape(nw, KT, 128, OUT_DIM)
                wparts.append(
                    np.ascontiguousarray(wsel[:, MF8:].transpose(2, 0, 1, 3))
                    .reshape(128, nw * (KT - MF8) * OUT_DIM)
                    .astype(bf16)
                )
                vparts.append(
                    np.ascontiguousarray(
                        (wsel[:, :MF8] * np.float32(WSC)).transpose(2, 0, 1, 3)
                    )
                    .reshape(128, nw * MF8 * OUT_DIM)
                    .astype(f8e4)
                )
            vpacked = np.concatenate(vparts, axis=1).view(bf16)
            m[f"c{ci // 2}"] = np.concatenate(xparts + wparts + [vpacked], axis=1)
        in_maps.append(m)

    res = run_bass_kernel_spmd(
        nc, in_maps, list(range(NCORES)), **_cache.get("runkw", {})
    )
    _cache["last_result"] = res

    out = np.empty((B, N, OUT_DIM), dtype=np.float32)
    for c in range(NCORES):
        off = 0
        cbins = core_bins[c]
        for ci in range(0, len(cbins), 2):
            bs = sum(len(gss) for b in cbins[ci : ci + 2] for _, gss in b)
            od = np.asarray(res.results[c][f"o{ci // 2}"], dtype=np.float32)
            out[core_samples[c][off : off + bs]] = od.reshape(
                OUT_DIM, bs, N
            ).transpose(1, 2, 0)
            off += bs
    out += np.asarray(bias, dtype=np.float32)[yi][:, None, :]
    return out
